# revision 26
# baseline (speedup 1.0000x reference)
"""Trainium2 Bass kernel for nn_AttentionSampling (sparse window attention block).

Sharding: 8 cores, data-parallel, 1024 windows (half a batch) per core; no
cross-core communication. All-bf16 matmuls (fp8/DoubleRow measured as a wash
on PE issue rate while costing error margin).

v4 design (vs 174us v1 baseline; measured 146.6us):
- v-projection is linear, so the windowed weighted-sum runs FIRST on raw
  `value` (DVE, windows on partitions) and only the 1024 downsampled tokens
  get projected (4x less PE work on the v path); the result moves to the
  transposed domain with DMA xbar transposes (no PE transposes, no
  PSUM->SBUF copies).
- LN1's affine folds into the FFN weights host-side (W1' = diag(g1) W1) and
  into the residual-2 accumulate (per-partition scalar), so the LN1 output
  is never materialized when ln1_b/ffn_b2/b_v are zero (a general program
  handles nonzero ones).
- Token range processed as chunks [512, 256, 256]: the big chunk keeps
  matmuls at N=512 (LDWEIGHTS fully hidden, 216ns/MM), the small tail
  chunks pipeline the serial LN/FFN dependency chain against remaining
  attention blocks.
- LN squares + applies on DVE (bf16 2x), stats matmuls bf16 with fp32 PSUM
  accumulation; relu epilogues on ACT; v loads and output stores on the
  GpSimd software DGE queue so the sync ring only carries q/k/weights and
  the xbar transposes.
"""

import sys
import types

try:
    import antenv.axon_hooks  # noqa: F401
except ImportError:
    _m = types.ModuleType("antenv.axon_hooks")
    _m.get_axon_ntff_profile_hook = lambda: None
    _m.set_axon_ntff_profile_hook = lambda h: None
    sys.modules["antenv.axon_hooks"] = _m
    try:
        import antenv

        antenv.axon_hooks = _m
    except ImportError:
        pass

import contextlib

import numpy as np

import concourse.bass as bass
import concourse.bacc as bacc_mod
import concourse.mybir as mybir
import concourse.tile as tile
from concourse.bass import ts, ds
from concourse.bass_utils import run_bass_kernel_spmd

FP32 = mybir.dt.float32
BF16 = mybir.dt.bfloat16
AF = mybir.ActivationFunctionType
OP = mybir.AluOpType

B, SQ, SK, D, F = 4, 2048, 8192, 512, 4
NCORES = 8
WPC = B * SQ // NCORES        # 1024 windows (= tokens) per core
KPC = WPC * F                 # 4096 keys per core
NBLK = WPC // 128             # 8 attention blocks: 128 windows / 512 keys
DT = D // 128                 # 4 d-tiles
EPS = 1e-5

# (col0, width, [blocks]) chunks of the token range
CHUNKS = [(0, 512, (0, 1, 2, 3)), (512, 256, (4, 5)), (768, 256, (6, 7))]

BIAS_NAMES = ["bq", "bk", "b1", "b2", "g1", "gb1", "g2", "gb2"]
BIX = {n: i for i, n in enumerate(BIAS_NAMES)}

_CACHE = {}


def build_program(general: bool):
    nc = bacc_mod.Bacc(None, target_bir_lowering=False)

    qT_d = nc.dram_tensor("qsw", [2, 128, DT * 512], BF16, kind="ExternalInput")
    kT_d = nc.dram_tensor("ksw", [NBLK // 2, 128, DT * 1024], BF16,
                          kind="ExternalInput")
    vN_d = nc.dram_tensor("vN", [KPC, D], BF16, kind="ExternalInput")
    wqk_d = nc.dram_tensor("wqk16", [2 * D, D], BF16, kind="ExternalInput")
    w12_d = nc.dram_tensor("w12_16", [2 * D, D], BF16, kind="ExternalInput")
    wv_d = nc.dram_tensor("wv16", [D, D], BF16, kind="ExternalInput")
    bias_d = nc.dram_tensor("biaspack", [128, len(BIAS_NAMES), DT], FP32,
                            kind="ExternalInput")
    mask_d = nc.dram_tensor("cmask", [128, 512], FP32, kind="ExternalInput")
    ident_d = nc.dram_tensor("cident", [128, 128], FP32, kind="ExternalInput")
    bvrow_d = nc.dram_tensor("bvrow16", [D], BF16, kind="ExternalInput")
    outT_d = nc.dram_tensor("outT", [D, WPC], BF16, kind="ExternalOutput")

    vN_t = vN_d.rearrange("(j t w f) d -> j w t (f d)", j=NBLK // 2, t=2, w=128)
    wqk_t = wqk_d.rearrange("(o p) n -> p o n", p=128)   # [128, 8, 512]
    w12_t = w12_d.rearrange("(o p) n -> p o n", p=128)
    wv_t = wv_d.rearrange("(o p) n -> p o n", p=128)
    outT_t = outT_d.rearrange("(o p) n -> p o n", p=128)

    with tile.TileContext(nc) as tc, contextlib.ExitStack() as ctx:
        singles = ctx.enter_context(tc.tile_pool(name="singles", bufs=1))
        kin_p = ctx.enter_context(tc.tile_pool(name="kin", bufs=3))
        vin_p = ctx.enter_context(tc.tile_pool(name="vin", bufs=3))
        ktp_p = ctx.enter_context(tc.tile_pool(name="ktp", bufs=2))
        att_p = ctx.enter_context(tc.tile_pool(name="att", bufs=3))
        aot_p = ctx.enter_context(tc.tile_pool(name="aot", bufs=2))
        sb_p = ctx.enter_context(tc.tile_pool(name="sbp", bufs=2))
        small = ctx.enter_context(tc.tile_pool(name="small", bufs=3))
        ps_proj = ctx.enter_context(tc.tile_pool(name="ps_proj", bufs=3, space="PSUM"))
        ps_sc = ctx.enter_context(tc.tile_pool(name="ps_sc", bufs=2, space="PSUM"))
        ps_st = ctx.enter_context(tc.tile_pool(name="ps_st", bufs=1, space="PSUM"))
        ps_bc = ctx.enter_context(tc.tile_pool(name="ps_bc", bufs=1, space="PSUM"))

        # ---- constants; ring order matters: q-proj's deps first, then k0 ----
        wqk = singles.tile([128, 2 * DT, 512], BF16, tag="wqk")
        nc.sync.dma_start(out=wqk[:, :DT, :], in_=wqk_t[:, :DT, :])      # wq
        q_ins = [singles.tile([128, DT, 512], BF16, tag=f"q_in{h}",
                              name=f"q_in{h}") for h in range(2)]
        nc.sync.dma_start(out=q_ins[0].rearrange("p a b -> p (a b)"), in_=qT_d[0])
        biasp = singles.tile([128, len(BIAS_NAMES), DT], FP32, tag="biasp")
        nc.sync.dma_start(out=biasp, in_=bias_d[:, :, :])
        nc.sync.dma_start(out=wqk[:, DT:, :], in_=wqk_t[:, DT:, :])      # wk

        ones_col = singles.tile([128, 1], BF16, tag="ones_col")
        nc.gpsimd.memset(ones_col, 1.0)
        ones_row = singles.tile([1, 128], FP32, tag="ones_row")
        nc.gpsimd.memset(ones_row, 1.0)
        eps_t = singles.tile([1, 1], FP32, tag="eps")
        nc.gpsimd.memset(eps_t, EPS)

        late = {}

        def load_early2():  # behind wq/q0/wk on the ring, before k1
            nc.sync.dma_start(out=q_ins[1].rearrange("p a b -> p (a b)"), in_=qT_d[1])
            t = singles.tile([128, 512], FP32, tag="mask")
            nc.sync.dma_start(out=t, in_=mask_d[:, :])
            late["mask"] = t
            t = singles.tile([128, DT, 512], BF16, tag="wv")
            nc.sync.dma_start(out=t, in_=wv_t)
            late["wv"] = t
            if general:
                t = singles.tile([128, 128], FP32, tag="ident")
                nc.sync.dma_start(out=t, in_=ident_d[:, :])
                late["ident"] = t
                t = singles.tile([1, 512], BF16, tag="bvrow")
                nc.gpsimd.dma_start(
                    out=t, in_=bass.AP(tensor=bvrow_d, offset=0, ap=[[0, 1], [1, 512]])
                )
                late["bvrow"] = t

        def load_late():
            t = singles.tile([128, 2 * DT, 512], BF16, tag="w12")
            nc.sync.dma_start(out=t, in_=w12_t)
            late["w12"] = t

        def bias_ap(name, dt_):
            return biasp[:, BIX[name], dt_ : dt_ + 1]

        qTp = singles.tile([128, DT, WPC], BF16, tag="qTp")

        def proj(w_sb, w_off, bias_name, in_sb, in_off, n, out_sb_ap):
            """out = relu(x @ W + b), epilogue on ACT."""
            for do in range(DT):
                ps = ps_proj.tile([128, 512], FP32, tag="proj_ps", name="proj_ps")
                ps = ps[:, :n]
                for ki in range(DT):
                    nc.tensor.matmul(
                        ps, lhsT=w_sb[:, w_off + ki, ts(do, 128)],
                        rhs=in_sb[:, ki, ds(in_off, n)],
                        start=(ki == 0), stop=(ki == DT - 1),
                    )
                nc.scalar.activation(
                    out=out_sb_ap(do), in_=ps, func=AF.Relu,
                    bias=bias_ap(bias_name, do), scale=1.0,
                )

        # ---- q projection, first half (only wq + q0 loaded yet) ----
        proj(wqk, 0, "bq", q_ins[0], 0, 512, lambda do: qTp[:, do, ds(0, 512)])

        # ---- attention blocks ----
        kv_tiles = [None, None]

        def emit_kv_load(j):  # k on the sync ring; v on the gpsimd queue
            k_in = kin_p.tile([128, DT, 1024], BF16, tag="k_in", name="k_in")
            nc.sync.dma_start(out=k_in.rearrange("p a b -> p (a b)"), in_=kT_d[j])
            v_in = vin_p.tile([128, 2, 2048], BF16, tag="v_in", name="v_in")
            nc.gpsimd.dma_start(out=v_in, in_=vN_t[j])
            kv_tiles[0], kv_tiles[1] = k_in, v_in

        def emit_block(b, aoT, load=True):
            t = b % 2
            if t == 0 and load:
                emit_kv_load(b // 2)
            k_in, v_in = kv_tiles

            kTp = ktp_p.tile([128, DT, 512], BF16, tag="kTp", name="kTp")
            proj(wqk, DT, "bk", k_in, t * 512, 512, lambda do: kTp[:, do, :])

            sc_ps = ps_sc.tile([128, 512], FP32, tag="sc_ps", name="sc_ps")
            for ki in range(DT):
                nc.tensor.matmul(
                    sc_ps, lhsT=qTp[:, ki, ts(b, 128)], rhs=kTp[:, ki, :],
                    start=(ki == 0), stop=(ki == DT - 1),
                )
            # band extraction: wts[p, f] = sc[p, 4p+f]
            sm = att_p.tile([128, 512], BF16, tag="sm", name="sm")
            nc.vector.tensor_tensor(sm, sc_ps, late["mask"], op=OP.mult)
            wts = small.tile([128, F], FP32, tag="wts", name="wts")
            nc.vector.tensor_reduce(
                out=wts, in_=sm.rearrange("p (kw f) -> p f kw", f=F),
                axis=mybir.AxisListType.X, op=OP.add,
            )
            # windowed downsample of raw value (bf16)
            ao = att_p.tile([128, 512], BF16, tag="ao", name="ao")
            vv = v_in[:, t, :]
            nc.vector.tensor_scalar(
                out=ao, in0=vv[:, ts(0, 512)], scalar1=wts[:, 0:1], scalar2=None,
                op0=OP.mult,
            )
            for f in range(1, F):
                nc.vector.scalar_tensor_tensor(
                    out=ao, in0=vv[:, ts(f, 512)], scalar=wts[:, f : f + 1],
                    in1=ao, op0=OP.mult, op1=OP.add,
                )
            if general:
                wsum = small.tile([128, 1], FP32, tag="wsum", name="wsum")
                nc.vector.tensor_reduce(
                    out=wsum, in_=wts, axis=mybir.AxisListType.X, op=OP.add
                )
                wsr_ps = ps_st.tile([1, 128], FP32, tag="wsr_ps", name="wsr_ps")
                nc.tensor.matmul(wsr_ps, lhsT=wsum, rhs=late["ident"],
                                 start=True, stop=True)
                nc.scalar.activation(
                    out=aoT["wsrow"][:, ts(aoT["bix"][b], 128)], in_=wsr_ps,
                    func=AF.Copy,
                )
            # move to transposed domain: aoT[p, o, w] = ao[w, 128o+p]
            # (scalar HWDGE queue: keeps transposes off the k-load ring)
            nc.scalar.dma_start_transpose(
                out=aoT["t"][:, :, ts(aoT["bix"][b], 128)], in_=ao
            )

        def emit_aot(blocks, width):
            t = aot_p.tile([128, DT, width], BF16, tag=f"aoT{width}", name="aoT")
            r = {"t": t, "bix": {b: i for i, b in enumerate(blocks)}}
            if general:
                r["wsrow"] = small.tile([1, width], BF16, tag=f"wsr{width}",
                                        name="wsrow")
            return r

        def emit_vproj_resid(col0, n, aoT):
            resid = sb_p.tile([128, DT, n], BF16, tag=f"resid{n}", name="resid")
            for do in range(DT):
                ps = ps_proj.tile([128, 512], FP32, tag="proj_ps", name="vproj_ps")
                ps = ps[:, :n]
                for ki in range(DT):
                    nc.tensor.matmul(
                        ps, lhsT=late["wv"][:, ki, ts(do, 128)], rhs=aoT["t"][:, ki, :],
                        start=(ki == 0),
                        stop=(ki == DT - 1 and not general),
                    )
                if general:
                    nc.tensor.matmul(
                        ps, lhsT=late["bvrow"][:, ts(do, 128)], rhs=aoT["wsrow"],
                        start=False, stop=True,
                    )
                nc.vector.tensor_tensor(
                    resid[:, do, :], ps, qTp[:, do, ds(col0, n)], op=OP.add
                )
            return resid

        def emit_ln(x_sb, n, out_cb):
            """LayerNorm over D of x_sb [128, DT, n] (bf16, transposed).
            out_cb(dt, y2) consumes normalized (pre-affine) tiles, which stay
            alive in the returned scratch tile."""
            scr = sb_p.tile([128, DT, n], BF16, tag=f"scr{n}", name="scr")
            nc.vector.tensor_tensor(
                scr.rearrange("p a b -> p (a b)"),
                x_sb.rearrange("p a b -> p (a b)"),
                x_sb.rearrange("p a b -> p (a b)"), op=OP.mult,
            )
            mean_ps = ps_st.tile([1, 512], FP32, tag="st_mean", name="st_mean")
            mean_ps = mean_ps[:, :n]
            for ki in range(DT):
                nc.tensor.matmul(
                    mean_ps, lhsT=ones_col, rhs=x_sb[:, ki, :],
                    start=(ki == 0), stop=(ki == DT - 1),
                )
            sq_ps = ps_st.tile([1, 512], FP32, tag="st_sq", name="st_sq")
            sq_ps = sq_ps[:, :n]
            for ki in range(DT):
                nc.tensor.matmul(
                    sq_ps, lhsT=ones_col, rhs=scr[:, ki, :],
                    start=(ki == 0), stop=(ki == DT - 1),
                )
            mean_sb = small.tile([1, 512], FP32, tag="mean_sb", name="mean_sb")
            mean_sb = mean_sb[:, :n]
            nc.scalar.activation(out=mean_sb, in_=mean_ps, func=AF.Copy, scale=1.0 / D)
            m2 = small.tile([1, 512], FP32, tag="m2", name="m2")
            m2 = m2[:, :n]
            nc.scalar.activation(out=m2, in_=mean_ps, func=AF.Square, scale=1.0 / D)
            var = small.tile([1, 512], FP32, tag="var", name="var")
            var = var[:, :n]
            nc.vector.scalar_tensor_tensor(
                out=var, in0=sq_ps, scalar=1.0 / D, in1=m2,
                op0=OP.mult, op1=OP.subtract,
            )
            nc.scalar.activation(out=var, in_=var, func=AF.Sqrt, bias=eps_t, scale=1.0)
            rstd_sb = small.tile([1, 512], FP32, tag="rstd_sb", name="rstd_sb")
            rstd_sb = rstd_sb[:, :n]
            nc.vector.reciprocal_approx_fast(out=rstd_sb, in_=var)

            bc = {}
            for nm, row in (("mu", mean_sb), ("rs", rstd_sb)):
                bps = ps_bc.tile([128, 512], FP32, tag="bc_ps", name="bc_" + nm)
                bps = bps[:, :n]
                nc.tensor.matmul(bps, lhsT=ones_row, rhs=row, start=True, stop=True)
                bsb = small.tile([128, 512], BF16, tag="bc_sb", name="bcs_" + nm)
                bsb = bsb[:, :n]
                nc.scalar.activation(out=bsb, in_=bps, func=AF.Copy)
                bc[nm] = bsb
            for dt_ in range(DT):
                y = scr[:, dt_, :]  # reuse squares tile as y2 output
                nc.vector.tensor_tensor(y, x_sb[:, dt_, :], bc["mu"], op=OP.subtract)
                nc.vector.tensor_tensor(y, y, bc["rs"], op=OP.mult)
                out_cb(dt_, y)
            return scr

        def emit_ln1(col0, n, resid):
            if general:
                xT = sb_p.tile([128, DT, n], BF16, tag=f"xT{n}", name="xT")

                def write_x(dt_, y):
                    nc.scalar.activation(
                        out=xT[:, dt_, :], in_=y, func=AF.Identity,
                        bias=bias_ap("gb1", dt_), scale=bias_ap("g1", dt_),
                    )
                y2 = emit_ln(resid, n, write_x)
                return {"y2": y2, "x": xT}
            y2 = emit_ln(resid, n, lambda dt_, y: None)
            return {"y2": y2, "x": y2}

        def emit_ffn(col0, n, st):
            hT = sb_p.tile([128, DT, n], BF16, tag=f"hT{n}", name="hT")
            proj(late["w12"], 0, "b1", st["x"], 0, n, lambda ht: hT[:, ht, :])

            resid2 = sb_p.tile([128, DT, n], BF16, tag=f"resid2_{n}", name="resid2")
            for do in range(DT):
                ps = ps_proj.tile([128, 512], FP32, tag="proj_ps", name="ffn2_ps")
                ps = ps[:, :n]
                for ki in range(DT):
                    nc.tensor.matmul(
                        ps, lhsT=late["w12"][:, DT + ki, ts(do, 128)],
                        rhs=hT[:, ki, :],
                        start=(ki == 0), stop=(ki == DT - 1),
                    )
                if general:
                    nc.vector.scalar_tensor_tensor(
                        out=resid2[:, do, :], in0=ps, scalar=bias_ap("b2", do),
                        in1=st["x"][:, do, :], op0=OP.add, op1=OP.add,
                    )
                else:
                    nc.vector.scalar_tensor_tensor(
                        out=resid2[:, do, :], in0=st["y2"][:, do, :],
                        scalar=bias_ap("g1", do), in1=ps, op0=OP.mult, op1=OP.add,
                    )
            return resid2

        def emit_ln2_out(col0, n, resid2, split_dma):
            out_sb = sb_p.tile([128, DT, n], BF16, tag=f"out{n}", name="out_sb")

            def write_out(dt_, y):
                nc.scalar.activation(
                    out=out_sb[:, dt_, :], in_=y, func=AF.Identity,
                    bias=bias_ap("gb2", dt_), scale=bias_ap("g2", dt_),
                )
                if split_dma:
                    nc.gpsimd.dma_start(
                        out=outT_t[:, dt_, ds(col0, n)], in_=out_sb[:, dt_, :]
                    )
            emit_ln(resid2, n, write_out)
            if not split_dma:
                nc.gpsimd.dma_start(out=outT_t[:, :, ds(col0, n)], in_=out_sb)

        # ---- main schedule ----
        c0, c1, c2 = CHUNKS
        aoT0 = emit_aot(c0[2], c0[1])
        emit_kv_load(0)
        load_early2()
        emit_block(0, aoT0, load=False)
        # q projection, second half (q1 now behind wk/k0 on the ring)
        proj(wqk, 0, "bq", q_ins[1], 0, 512, lambda do: qTp[:, do, ds(512, 512)])
        emit_block(1, aoT0)
        load_late()
        emit_block(2, aoT0)
        emit_block(3, aoT0)
        r0 = emit_vproj_resid(c0[0], c0[1], aoT0)
        aoT1 = emit_aot(c1[2], c1[1])
        emit_block(4, aoT1)
        st0 = emit_ln1(c0[0], c0[1], r0)
        emit_block(5, aoT1)
        r1 = emit_vproj_resid(c1[0], c1[1], aoT1)
        aoT2 = emit_aot(c2[2], c2[1])
        emit_block(6, aoT2)
        rr0 = emit_ffn(c0[0], c0[1], st0)
        emit_block(7, aoT2)
        st1 = emit_ln1(c1[0], c1[1], r1)
        r2 = emit_vproj_resid(c2[0], c2[1], aoT2)
        emit_ln2_out(c0[0], c0[1], rr0, split_dma=False)
        rr1 = emit_ffn(c1[0], c1[1], st1)
        st2 = emit_ln1(c2[0], c2[1], r2)
        emit_ln2_out(c1[0], c1[1], rr1, split_dma=False)
        rr2 = emit_ffn(c2[0], c2[1], st2)
        emit_ln2_out(c2[0], c2[1], rr2, split_dma=True)

    nc.finalize()
    return nc


def _band_mask():
    p = np.arange(128)[:, None]
    k = np.arange(512)[None, :]
    band = (k - 4 * p >= 0) & (k - 4 * p <= 3)
    return band.astype(np.float32)


def _is_general(inputs):
    f32 = lambda n: np.asarray(inputs[n], dtype=np.float32)
    return bool(
        np.any(f32("b_v")) or np.any(f32("ln1_b")) or np.any(f32("ffn_b2"))
    )


def _host_prep(inputs, general):
    """Shared (per-core-invariant) tensors, host-side precompute."""
    import ml_dtypes

    BF = ml_dtypes.bfloat16
    f32 = lambda x: np.asarray(x, dtype=np.float32)

    def colpack(v):  # [D] -> [128, DT] column tile layout (d = o*128 + p)
        return f32(v).reshape(DT, 128).T

    wq, wk, wv = f32(inputs["w_q"]), f32(inputs["w_k"]), f32(inputs["w_v"])
    w1, w2 = f32(inputs["ffn_w1"]), f32(inputs["ffn_w2"])
    g1 = f32(inputs["ln1_g"])
    if not general:
        w1 = w1 * g1[:, None]  # fold LN1 gain into FFN1 (gb1 == 0)

    shared = {
        "wqk16": np.ascontiguousarray(np.concatenate([wq, wk], axis=0)).astype(BF),
        "w12_16": np.ascontiguousarray(np.concatenate([w1, w2], axis=0)).astype(BF),
        "wv16": np.ascontiguousarray(wv).astype(BF),
        "cident": np.eye(128, dtype=np.float32),
        "cmask": _band_mask(),
        "bvrow16": f32(inputs["b_v"]).astype(BF),
    }
    bias_cols = {
        "bq": f32(inputs["b_q"]), "bk": f32(inputs["b_k"]),
        "b1": f32(inputs["ffn_b1"]), "b2": f32(inputs["ffn_b2"]),
        "g1": g1, "gb1": f32(inputs["ln1_b"]),
        "g2": f32(inputs["ln2_g"]), "gb2": f32(inputs["ln2_b"]),
    }
    bp = np.stack([colpack(bias_cols[n]) for n in BIAS_NAMES], axis=1)
    shared["biaspack"] = np.ascontiguousarray(bp)  # [128, NB, DT]
    return shared


def core_inputs(query, key_t, value, c):
    """Per-core input tensors."""
    import ml_dtypes

    BF = ml_dtypes.bfloat16
    bi, half = c // 2, c % 2
    w0 = half * WPC
    qT = query[bi, w0 : w0 + WPC, :].T
    kT = key_t[bi, w0 * F : (w0 + WPC) * F, :].T
    qs = qT.reshape(DT, 128, WPC)
    qsw = np.stack([
        qs[:, :, h * 512 : (h + 1) * 512].transpose(1, 0, 2).reshape(128, -1)
        for h in range(2)
    ])
    ks = kT.reshape(DT, 128, KPC)
    ksw = np.stack([
        ks[:, :, j * 1024 : (j + 1) * 1024].transpose(1, 0, 2).reshape(128, -1)
        for j in range(NBLK // 2)
    ])
    return {
        "qsw": np.ascontiguousarray(qsw).astype(BF),
        "ksw": np.ascontiguousarray(ksw).astype(BF),
        "vN": np.ascontiguousarray(
            value[bi, w0 * F : (w0 + WPC) * F, :]).astype(BF),
    }


def gather_out(results):
    """[WPC, D] fp32 from one core's outputs."""
    return results["outT"].astype(np.float32).T


def kernel(**inputs):
    general = _is_general(inputs)
    key_ = ("prog", general)
    if key_ not in _CACHE:
        _CACHE[key_] = build_program(general)
    nc = _CACHE[key_]

    shared = _host_prep(inputs, general)
    query = np.asarray(inputs["query"], dtype=np.float32)
    key_t = np.asarray(inputs["key"], dtype=np.float32)
    value = np.asarray(inputs["value"], dtype=np.float32)

    in_maps = []
    for c in range(NCORES):
        m = dict(shared)
        m.update(core_inputs(query, key_t, value, c))
        in_maps.append(m)

    res = run_bass_kernel_spmd(nc, in_maps, core_ids=list(range(NCORES)))
    _CACHE["last_result"] = res
    out = np.empty((B, SQ, D), dtype=np.float32)
    for c in range(NCORES):
        bi, half = c // 2, c % 2
        w0 = half * WPC
        out[bi, w0 : w0 + WPC, :] = gather_out(res.results[c])
    return out
